# revision 1
# baseline (speedup 1.0000x reference)
"""AttnBlock (GroupNorm + single-head self-attention + proj + residual) on 8 trn2 cores.

Sharding: core = (batch b = core//4, query-block qb = core%4). Each core gets its
batch's x rolled so its 1024 queries are columns 0:1024; attention key/value
order is permutation-invariant so the roll is free. No cross-core communication.

Math (validated in numpy to 6e-8 rel err):
  GroupNorm folded into per-channel affine A, B applied to the weights:
    hn = A*x + B (per channel)
    q  = (wq*A) @ x + (wq@B + bq)
    k-bias drops (softmax shift invariance); v/o biases collapse to
    bo'' = wo@(wv@B + bv) + bo added at the end.
  logitsT[j,i] = sum_ci x[ci,j] * (A[ci] * (wk^T q)[ci,i])   (keys-major layout,
    so softmax reduction is a ones-matmul and no transposes are ever needed)
  P = exp(logitsT/sqrt(C)) unnormalized; o = (wv*A@x) @ P; the division by the
  column sums is applied to the projection output (it commutes with wo@).

All heavy matmuls run as float32r (full PE rate at free dim 512); tiles feeding
the PE are declared float32r so every producer satisfies the BIR rounding rule,
and f32-only consumers (reductions, residual add, bias matmuls) read via bitcast.

Schedule notes (round 2, from perfetto analysis of the 304us v1):
  - DMA issue order: tiny vectors first, then x tiles, then weights — the
    group-selector used by the very first stats matmul otherwise lands last.
  - PE warm-up matmuls on a zero tile keep the HAM clock-gate at 2.4 GHz
    through the prologue so the real matmul stream starts warm.
  - GroupNorm stats: sum(x) on DVE (tensor_reduce, 2x mode) + sum(x^2) on the
    otherwise-idle ACT (Square + accum_out into the q scratch); all Squares
    emitted before all Sqrts so the ACT table loads exactly twice.
  - Chunk epilogue: plain o copies -> projection immediately; 1/s broadcast
    happens in parallel and is applied in the final output DVE op.
"""

import numpy as np

import concourse.bass as bass
import concourse.bacc as bacc
import concourse.tile as tile
from concourse import mybir
from concourse.bass_utils import run_bass_kernel_spmd

F32 = mybir.dt.float32
F32R = mybir.dt.float32r
AF = mybir.ActivationFunctionType
ALU = mybir.AluOpType
AX = mybir.AxisListType

B, C, HH, WW = 2, 512, 64, 64
N = HH * WW          # 4096 pixels
NQ = N // 4          # queries per core
G = 32               # groups
GPT = 8              # groups per 128-channel tile
NT = C // 128        # 4 channel tiles
JT = N // 128        # 32 key tiles
CW = 512             # query chunk width
NCH = NQ // CW       # 2 chunks per core
EPS = 1e-6
SCALE = float(C) ** -0.5
GDIV = 1.0 / 16.0  # st2 carries per-channel means; groups have 16 channels

_CACHE: dict = {}


def _f32(ap):
    return ap.bitcast(F32)


def _build_bass():
    nc = bacc.Bacc("TRN2")

    warm_d = nc.declare_dram_parameter("warm", [128, 128], F32, isOutput=False)
    x_d = nc.declare_dram_parameter("x", [C, N], F32R, isOutput=False)
    wqT_d = nc.declare_dram_parameter("wqT", [C, C], F32R, isOutput=False)
    wk_d = nc.declare_dram_parameter("wk", [C, C], F32R, isOutput=False)
    wvT_d = nc.declare_dram_parameter("wvT", [C, C], F32R, isOutput=False)
    woT_d = nc.declare_dram_parameter("woT", [C, C], F32R, isOutput=False)
    gnw_d = nc.declare_dram_parameter("gnw", [C], F32, isOutput=False)
    gnb_d = nc.declare_dram_parameter("gnb", [C], F32, isOutput=False)
    bq_d = nc.declare_dram_parameter("bq", [C], F32, isOutput=False)
    bv_d = nc.declare_dram_parameter("bv", [C], F32, isOutput=False)
    bo_d = nc.declare_dram_parameter("bo", [C], F32, isOutput=False)
    sel_d = nc.declare_dram_parameter("sel", [128, GPT], F32, isOutput=False)
    selT_d = nc.declare_dram_parameter("selT", [GPT, 128], F32, isOutput=False)
    out_d = nc.declare_dram_parameter("out", [C, NQ], F32, isOutput=True)

    dram = dict(warm=warm_d, x=x_d, wqT=wqT_d, wk=wk_d, wvT=wvT_d, woT=woT_d,
                gnw=gnw_d, gnb=gnb_d, bq=bq_d, bv=bv_d, bo=bo_d,
                sel=sel_d, selT=selT_d, out=out_d)
    with tile.TileContext(nc) as tc, \
         nc.allow_low_precision(reason="float32r tiles are 4-byte fp32 feeding the PE"):
        _emit(tc, {k: v.ap() for k, v in dram.items()})
    nc.compile()
    return nc


def _emit(tc, d):
    nc = tc.nc

    # ---- long-lived pools -------------------------------------------------
    xp = tc.alloc_tile_pool(name="xp", bufs=NT)
    wp = tc.alloc_tile_pool(name="wp", bufs=NT)        # wk, woT (per tag)
    wearly = tc.alloc_tile_pool(name="wear", bufs=NT)  # wqT, wvT
    vecs = tc.alloc_tile_pool(name="vecs", bufs=1)
    qp = tc.alloc_tile_pool(name="qp", bufs=NT)
    vtp = tc.alloc_tile_pool(name="vtp", bufs=JT)

    # ---- DMA in (order matters: small tensors first, x before weights) ----
    warm_sb = vecs.tile([128, 128], F32, tag="warm")
    nc.sync.dma_start(out=warm_sb[:, :], in_=d["warm"])
    sel_sb = vecs.tile([128, GPT], F32, tag="sel")
    nc.sync.dma_start(out=sel_sb[:, :], in_=d["sel"])
    selT_sb = vecs.tile([GPT, 128], F32, tag="selT")
    nc.sync.dma_start(out=selT_sb[:, :], in_=d["selT"])

    def load_vec(name, tag):
        vt = vecs.tile([128, NT], F32, tag=tag)
        nc.sync.dma_start(out=vt[:, :], in_=d[name].rearrange("(t p) -> p t", p=128))
        return vt

    gnw_sb = load_vec("gnw", "gnw")
    gnb_sb = load_vec("gnb", "gnb")
    bqv_sb = load_vec("bq", "bqv")
    bvv_sb = load_vec("bv", "bvv")
    bov_sb = load_vec("bo", "bov")

    x_sb = []
    XSP = 2  # x DMA split factor per tile
    XW = N // XSP
    x_t = d["x"].rearrange("(t p) n -> t p n", p=128)
    for t in range(NT):
        xt = xp.tile([128, N], F32R, tag="x", name=f"xt{t}")
        for hh in range(XSP):
            nc.sync.dma_start(out=xt[:, hh * XW:(hh + 1) * XW],
                              in_=x_t[t][:, hh * XW:(hh + 1) * XW])
        x_sb.append(xt)

    def load_w(pool, name, tag):
        tiles = []
        r = d[name].rearrange("(t p) m -> t p m", p=128)
        for t in range(NT):
            wt = pool.tile([128, C], F32R, tag=tag)
            nc.sync.dma_start(out=wt[:, :], in_=r[t])
            tiles.append(wt)
        return tiles

    wqT_sb = load_w(wearly, "wqT", "wqT")
    wvT_sb = load_w(wearly, "wvT", "wvT")
    wk_sb = load_w(wp, "wk", "wk")
    woT_sb = load_w(wp, "woT", "woT")

    ones32_sb = vecs.tile([128, 128], F32, tag="ones32")
    nc.vector.memset(ones32_sb[:, :], 1.0)
    ones128_sb = vecs.tile([128, 128], F32R, tag="ones128")
    nc.vector.tensor_copy(out=ones128_sb[:, :], in_=ones32_sb[:, :])

    A_sb = vecs.tile([128, NT], F32, tag="A")
    B_sb = vecs.tile([128, NT], F32, tag="B")
    bqp_sb = vecs.tile([128, NT], F32, tag="bqp")
    bvp_sb = vecs.tile([128, NT], F32, tag="bvp")
    bop_sb = vecs.tile([128, NT], F32, tag="bop")

    # q tiles double as ACT scratch for the Square pass during stats
    q_sb = [qp.tile([128, NQ], F32R, tag="q", name=f"q{i}") for i in range(4)]

    # ---- GroupNorm stats → per-channel affine A, B ------------------------
    with tc.tile_pool(name="stp", bufs=4) as stp, \
         tc.tile_pool(name="pssm", bufs=2, space="PSUM") as ps_sm:
        nwarm = [0]

        def emit_warm(n):
            for _ in range(n):
                wt = ps_sm.tile([128, 128], F32, tag="warm", name=f"wm{nwarm[0]}")
                nwarm[0] += 1
                nc.tensor.matmul(out=wt[:, :], lhsT=warm_sb[:, 0:128],
                                 rhs=warm_sb[:, :], start=True, stop=True)

        emit_warm(12)
        gps_t = []
        for t in range(NT):
            st2 = stp.tile([128, 2], F32, tag="st2", name=f"st2_{t}")
            if t < NT - 1:
                # DVE bn_stats per half as the DMA lands
                st = stp.tile([128, 8, 6], F32, tag="bnst", name=f"bnst{t}")
                xr = _f32(x_sb[t][:, :]).rearrange("p (s n) -> p s n", s=8)
                for s in range(8):
                    nc.vector.bn_stats(out=st[:, s, :], in_=xr[:, s, :])
                mv = stp.tile([128, 2], F32, tag="mv", name=f"mv{t}")
                nc.vector.bn_aggr(out=mv[:, :], in_=st[:, :, :])
                nc.vector.tensor_copy(out=st2[:, 0:1], in_=mv[:, 0:1])
                nc.vector.tensor_mul(out=st2[:, 1:2], in0=mv[:, 0:1], in1=mv[:, 0:1])
                nc.vector.tensor_add(out=st2[:, 1:2], in0=st2[:, 1:2], in1=mv[:, 1:2])
            else:
                # last tile: sum(x^2) on ACT (Square+accum into q scratch),
                # sum(x) on DVE — the two engines run in parallel
                sq4 = stp.tile([128, NT], F32, tag="sq4", name=f"sq4_{t}")
                for k in range(4):
                    nc.scalar.activation(out=q_sb[k][:, :],
                                         in_=x_sb[t][:, k * NQ:(k + 1) * NQ],
                                         func=AF.Square, bias=0.0, scale=1.0,
                                         accum_out=sq4[:, k:k + 1])
                nc.vector.tensor_reduce(out=st2[:, 0:1], in_=_f32(x_sb[t][:, :]),
                                        axis=AX.X, op=ALU.add)
                nc.vector.tensor_scalar_mul(out=st2[:, 0:1], in0=st2[:, 0:1],
                                            scalar1=1.0 / N)
                nc.vector.tensor_reduce(out=st2[:, 1:2], in_=sq4[:, :],
                                        axis=AX.X, op=ALU.add)
                nc.vector.tensor_scalar_mul(out=st2[:, 1:2], in0=st2[:, 1:2],
                                            scalar1=1.0 / N)
            gps = ps_sm.tile([GPT, 2], F32, tag="gps", name=f"gps{t}")
            nc.tensor.matmul(out=gps[:, :], lhsT=sel_sb[:, :], rhs=st2[:, :],
                             start=True, stop=True)
            gps_t.append(gps)
            emit_warm((10, 10, 6, 0)[t])

        # group mean / rstd; all DVE preps first, then batched ACT Sqrts
        grp_t = []
        for t in range(NT):
            grp = stp.tile([GPT, 2], F32, tag="grp", name=f"grp{t}")
            nc.vector.tensor_scalar_mul(out=grp[:, :], in0=gps_t[t][:, :], scalar1=GDIV)
            gtmp = stp.tile([GPT, 1], F32, tag="gtmp", name=f"gtmp{t}")
            nc.vector.tensor_mul(out=gtmp[:, :], in0=grp[:, 0:1], in1=grp[:, 0:1])
            nc.vector.tensor_sub(out=grp[:, 1:2], in0=grp[:, 1:2], in1=gtmp[:, :])
            nc.vector.tensor_scalar_add(out=grp[:, 1:2], in0=grp[:, 1:2], scalar1=EPS)
            grp_t.append(grp)
        for t in range(NT):
            nc.scalar.activation(out=grp_t[t][:, 1:2], in_=grp_t[t][:, 1:2],
                                 func=AF.Sqrt, bias=0.0, scale=1.0)
        emit_warm(2)
        for t in range(NT):
            nc.vector.reciprocal(out=grp_t[t][:, 1:2], in_=grp_t[t][:, 1:2])
            mrp = ps_sm.tile([128, 2], F32, tag="sm", name=f"mrp{t}")
            nc.tensor.matmul(out=mrp[:, :], lhsT=selT_sb[:, :], rhs=grp_t[t][:, :],
                             start=True, stop=True)
            tcol = slice(t, t + 1)
            nc.vector.tensor_mul(out=A_sb[:, tcol], in0=gnw_sb[:, tcol], in1=mrp[:, 1:2])
            nc.vector.tensor_mul(out=B_sb[:, tcol], in0=mrp[:, 0:1], in1=A_sb[:, tcol])
            nc.vector.tensor_sub(out=B_sb[:, tcol], in0=gnb_sb[:, tcol], in1=B_sb[:, tcol])

    ps_mm = tc.alloc_tile_pool(name="psmm", bufs=3, space="PSUM")

    # ---- folded biases (need un-scaled wqT/wvT, so run before scaling) ----
    for ot in range(4):
        ocol = slice(ot, ot + 1)
        bps = ps_mm.tile([128, 1], F32, tag="mm", name=f"bq{ot}")
        for ci in range(NT):
            nc.tensor.matmul(out=bps[:, :],
                             lhsT=_f32(wqT_sb[ci][:, ot * 128:(ot + 1) * 128]),
                             rhs=B_sb[:, ci:ci + 1],
                             start=(ci == 0), stop=(ci == NT - 1))
        nc.vector.tensor_add(out=bqp_sb[:, ocol], in0=bps[:, :], in1=bqv_sb[:, ocol])
    for ot in range(4):
        ocol = slice(ot, ot + 1)
        bps2 = ps_mm.tile([128, 1], F32, tag="mm", name=f"bv{ot}")
        for ci in range(NT):
            nc.tensor.matmul(out=bps2[:, :],
                             lhsT=_f32(wvT_sb[ci][:, ot * 128:(ot + 1) * 128]),
                             rhs=B_sb[:, ci:ci + 1],
                             start=(ci == 0), stop=(ci == NT - 1))
        nc.vector.tensor_add(out=bvp_sb[:, ocol], in0=bps2[:, :], in1=bvv_sb[:, ocol])

    # ---- scale wq^T / wv^T rows by A, then q = wqA @ x[:, 0:NQ] + bq' -----
    for t in range(NT):
        nc.vector.tensor_scalar_mul(out=wqT_sb[t][:, :], in0=_f32(wqT_sb[t][:, :]),
                                    scalar1=A_sb[:, t:t + 1])
    for t in range(NT):
        nc.vector.tensor_scalar_mul(out=wvT_sb[t][:, :], in0=_f32(wvT_sb[t][:, :]),
                                    scalar1=A_sb[:, t:t + 1])
    for ot in range(4):
        for ch in range(NCH):
            csl = slice(ch * CW, (ch + 1) * CW)
            qps = ps_mm.tile([128, CW], F32, tag="mm")
            for ci in range(NT):
                nc.tensor.matmul(out=qps[:, :],
                                 lhsT=wqT_sb[ci][:, ot * 128:(ot + 1) * 128],
                                 rhs=x_sb[ci][:, csl],
                                 start=(ci == 0), stop=(ci == NT - 1))
            nc.vector.tensor_scalar_add(out=q_sb[ot][:, csl], in0=qps[:, :],
                                        scalar1=bqp_sb[:, ot:ot + 1])

    ps_o = tc.alloc_tile_pool(name="pso", bufs=4, space="PSUM")

    # ---- vT[j, c] = ((wv*A) @ x)^T ----------------------------------------
    vt_sb = []
    for jt in range(JT):
        jsl = slice(jt * 128, (jt + 1) * 128)
        vps = ps_mm.tile([128, C], F32, tag="mm")
        for ci in range(NT):
            nc.tensor.matmul(out=vps[:, :], lhsT=x_sb[ci][:, jsl],
                             rhs=wvT_sb[ci][:, :],
                             start=(ci == 0), stop=(ci == NT - 1))
        vt = vtp.tile([128, C], F32R, tag="vt")
        nc.vector.tensor_copy(out=vt[:, :], in_=vps[:, :])
        vt_sb.append(vt)

    # ---- bo'' = wo@bv' + bo (emitted here so it never waits on the late woT DMA)
    for ot in range(4):
        ocol = slice(ot, ot + 1)
        bps3 = ps_mm.tile([128, 1], F32, tag="mm", name=f"bo{ot}")
        for ci in range(NT):
            nc.tensor.matmul(out=bps3[:, :],
                             lhsT=_f32(woT_sb[ci][:, ot * 128:(ot + 1) * 128]),
                             rhs=bvp_sb[:, ci:ci + 1],
                             start=(ci == 0), stop=(ci == NT - 1))
        nc.vector.tensor_add(out=bop_sb[:, ocol], in0=bps3[:, :], in1=bov_sb[:, ocol])

    # ---- attention chunks -------------------------------------------------
    qkp = tc.alloc_tile_pool(name="qkp", bufs=NT)
    pp = tc.alloc_tile_pool(name="pp", bufs=2)
    osb = tc.alloc_tile_pool(name="osb", bufs=4)
    outp = tc.alloc_tile_pool(name="outp", bufs=2)
    smsb = tc.alloc_tile_pool(name="smsb", bufs=1)

    for ch in range(NCH):
        csl = slice(ch * CW, (ch + 1) * CW)
        # qk[ci, i] = A[ci] * (wk^T q)[ci, i]
        qk_sb = []
        for ci in range(NT):
            kps = ps_mm.tile([128, CW], F32, tag="mm")
            for ot in range(4):
                nc.tensor.matmul(out=kps[:, :],
                                 lhsT=wk_sb[ot][:, ci * 128:(ci + 1) * 128],
                                 rhs=q_sb[ot][:, csl],
                                 start=(ot == 0), stop=(ot == NT - 1))
            qk = qkp.tile([128, CW], F32R, tag="qk")
            nc.vector.tensor_scalar_mul(out=qk[:, :], in0=kps[:, :],
                                        scalar1=A_sb[:, ci:ci + 1])
            qk_sb.append(qk)

        o_ps = [ps_o.tile([128, CW], F32, tag="o", name=f"o{ch}_{i}") for i in range(4)]
        sacc = smsb.tile([128, CW], F32R, tag="sacc", name=f"sacc{ch}")
        for jt in range(JT):
            jsl = slice(jt * 128, (jt + 1) * 128)
            lps = ps_mm.tile([128, CW], F32, tag="mm")
            for ci in range(NT):
                nc.tensor.matmul(out=lps[:, :], lhsT=x_sb[ci][:, jsl],
                                 rhs=qk_sb[ci][:, :],
                                 start=(ci == 0), stop=(ci == NT - 1))
            P = pp.tile([128, CW], F32R, tag="P")
            nc.scalar.activation(out=P[:, :], in_=lps[:, :], func=AF.Exp,
                                 bias=0.0, scale=SCALE)
            for co in range(4):
                nc.tensor.matmul(out=o_ps[co][:, :],
                                 lhsT=vt_sb[jt][:, co * 128:(co + 1) * 128],
                                 rhs=P[:, :],
                                 start=(jt == 0), stop=(jt == JT - 1),
                                 skip_group_check=True)
            if jt == 0:
                nc.vector.tensor_copy(out=sacc[:, :], in_=_f32(P[:, :]))
            else:
                nc.vector.tensor_add(out=sacc[:, :], in0=_f32(sacc[:, :]),
                                     in1=_f32(P[:, :]))

        # epilogue: plain o copies -> project immediately; 1/s broadcast in
        # parallel; normalize + bias + residual fused in the final DVE ops.
        last = ch == NCH - 1
        if last:
            # tail chunk: normalize during the PSUM->SBUF copy so the final
            # DVE chain is 2 ops; costs a small PE stall waiting for 1/s
            rbp = ps_mm.tile([128, CW], F32, tag="mm")
            nc.tensor.matmul(out=rbp[:, :], lhsT=ones128_sb[:, :], rhs=sacc[:, :],
                             start=True, stop=True)
            rsb = smsb.tile([128, CW], F32, tag="rsb")
            nc.vector.reciprocal_approx_fast(out=rsb[:, :], in_=rbp[:, :])
        o_sb = []
        for co in range(4):
            ot_ = osb.tile([128, CW], F32R, tag="osb")
            if last:
                nc.vector.tensor_mul(out=ot_[:, :], in0=o_ps[co][:, :], in1=rsb[:, :])
            else:
                nc.vector.tensor_copy(out=ot_[:, :], in_=o_ps[co][:, :])
            o_sb.append(ot_)
        prp_t = []
        for co in range(4):
            prp = ps_o.tile([128, CW], F32, tag="o", name=f"pr{ch}_{co}")
            for c in range(NT):
                nc.tensor.matmul(out=prp[:, :],
                                 lhsT=woT_sb[c][:, co * 128:(co + 1) * 128],
                                 rhs=o_sb[c][:, :],
                                 start=(c == 0), stop=(c == NT - 1))
            prp_t.append(prp)
        if not last:
            rbp = ps_mm.tile([128, CW], F32, tag="mm")
            nc.tensor.matmul(out=rbp[:, :], lhsT=ones128_sb[:, :], rhs=sacc[:, :],
                             start=True, stop=True)
            rsb = smsb.tile([128, CW], F32, tag="rsb")
            nc.vector.reciprocal_approx_fast(out=rsb[:, :], in_=rbp[:, :])
        for co in range(4):
            ou = outp.tile([128, CW], F32, tag="out")
            if last:
                nc.vector.tensor_scalar_add(out=ou[:, :], in0=prp_t[co][:, :],
                                            scalar1=bop_sb[:, co:co + 1])
            else:
                nc.vector.tensor_mul(out=ou[:, :], in0=prp_t[co][:, :], in1=rsb[:, :])
                nc.vector.tensor_scalar_add(out=ou[:, :], in0=ou[:, :],
                                            scalar1=bop_sb[:, co:co + 1])
            nc.vector.tensor_add(out=ou[:, :], in0=ou[:, :],
                                 in1=_f32(x_sb[co][:, csl]))
            nc.sync.dma_start(out=d["out"][co * 128:(co + 1) * 128, csl], in_=ou[:, :])

    for p in (smsb, outp, osb, pp, qkp, ps_o, ps_mm, vtp, qp, vecs,
              wearly, wp, xp):
        p.release()


def _sel_consts():
    sel = np.zeros((128, GPT), np.float32)
    for p in range(128):
        sel[p, p // 16] = 1.0
    return sel, np.ascontiguousarray(sel.T)


def kernel(x, gn_w, gn_b, wq, bq, wk, bk, wv, bv, wo, bo):
    del bk  # exactly cancelled by softmax shift invariance
    if "nc" not in _CACHE:
        _CACHE["nc"] = _build_bass()
    nc = _CACHE["nc"]

    x = np.ascontiguousarray(np.asarray(x, np.float32)).reshape(B, C, N)
    wqT = np.ascontiguousarray(np.asarray(wq, np.float32).T)
    wkn = np.ascontiguousarray(np.asarray(wk, np.float32))
    wvT = np.ascontiguousarray(np.asarray(wv, np.float32).T)
    woT = np.ascontiguousarray(np.asarray(wo, np.float32).T)
    vecs = {n: np.ascontiguousarray(np.asarray(v, np.float32))
            for n, v in (("gnw", gn_w), ("gnb", gn_b), ("bq", bq), ("bv", bv),
                         ("bo", bo))}
    sel, selT = _sel_consts()
    warm = np.zeros((128, 128), np.float32)

    in_maps = []
    for core in range(8):
        b, qb = core // 4, core % 4
        xb = np.ascontiguousarray(np.roll(x[b], -qb * NQ, axis=1))
        in_maps.append({"x": xb, "wqT": wqT, "wk": wkn, "wvT": wvT, "woT": woT,
                        "sel": sel, "selT": selT, "warm": warm, **vecs})

    _CACHE["last_in_maps"] = in_maps
    res = run_bass_kernel_spmd(nc, in_maps, list(range(8))).results
    out = np.empty((B, C, N), np.float32)
    for core in range(8):
        b, qb = core // 4, core % 4
        out[b][:, qb * NQ:(qb + 1) * NQ] = res[core]["out"]
    return out.reshape(B, C, HH, WW)



# revision 6
# speedup vs baseline: 1.0297x; 1.0297x over previous
"""AttnBlock (GroupNorm + single-head self-attention + proj + residual) on 8 trn2 cores.

Sharding: core = (batch b = core//4, query-block qb = core%4). Each core gets its
batch's x rolled so its 1024 queries are columns 0:1024; attention key/value
order is permutation-invariant so the roll is free. No cross-core communication.

Math (validated in numpy to 7e-8 rel err): the four 1x1 convs are fused on the
HOST into two C*C matrices (pure weight preprocessing, no data dependence):
    K2  = wq^T @ wk          (logits bilinear form:  l[i,j] = hn_i^T K2 hn_j)
    W3  = wo @ wv            (value+proj fused:      out_pre = W3 @ hn @ P)
GroupNorm folds into a per-channel affine hn = A*x + B on device:
    qk2[b,i] = A[b] * (sum_a A[a] K2[a,b] x[a,i]) + A[b]*cb[b],
      cb = K2^T B + wk^T bq;  the pure-B logits terms are constant per query
      and cancel in softmax, exactly like the k-bias.
    logitsT[j,i] = sum_b x[b,j] qk2[b,i]   (keys-major, no transposes)
    P = exp(logitsT/sqrt(C)) unnormalized; o_pre = ut^T @ P with
    ut[j,co] = sum_b A[b] W3T[b,co] x[b,j]; out = o_pre/s + fb + x where
    fb = W3^T... (W3T^T B + wo@bv + bo) and s = column sums of P.

Schedule (from perfetto analysis of the 262us baseline):
  - DMA order: vp/selT (tiny), x (10 pieces), K2, W3T. All on the sync queue
    FIFO; x lands ~21us, stats chain done ~25us, main loop starts ~28us.
  - PE warm-up matmuls on a memset ones tile start at ~0.8us (no DMA!) and are
    re-armed by matmuls reading each landed x piece, so the HAM clock gate
    never sees a 3.4us idle window (which would halve the PE clock).
  - Main loop per key tile: 4 logits mm, exp on ACT, 4 ut mm (chunk 0 only),
    4 o mms lagged one iteration behind (hides exp latency), running-sum of P
    on DVE, ut PSUM->SBUF drain on ACT.
  - Softmax denominator: sum P(jt<=30) on DVE + final P31 via a second
    accumulating ones-matmul, so 1/s is ready before the last o mm finishes.
  - Epilogue per chunk: DVE mul by 1/s (frees PSUM banks asap), residual+bias
    add on GpSimd (xq = x + fb precomputed there too), DMA out per 128-row
    block as it completes.
"""

import numpy as np

import concourse.bass as bass
import concourse.bacc as bacc
import concourse.tile as tile
from concourse import mybir
from concourse.bass_utils import run_bass_kernel_spmd

F32 = mybir.dt.float32
F32R = mybir.dt.float32r
AF = mybir.ActivationFunctionType
ALU = mybir.AluOpType
AX = mybir.AxisListType

B, C, HH, WW = 2, 512, 64, 64
N = HH * WW          # 4096 pixels
NQ = N // 4          # queries per core
G = 32               # groups
GPT = 8              # groups per 128-channel tile
NT = C // 128        # 4 channel tiles
JT = N // 128        # 32 key tiles
CW = 512             # query chunk width
NCH = NQ // CW       # 2 chunks per core
EPS = 1e-6
SCALE = float(C) ** -0.5
GDIV = 1.0 / 16.0    # 16 channels per group

_CACHE: dict = {}


def _f32(ap):
    return ap.bitcast(F32)


def _build_bass():
    nc = bacc.Bacc("TRN2")

    x_d = nc.declare_dram_parameter("x", [C, N], F32R, isOutput=False)
    k2_d = nc.declare_dram_parameter("K2", [C, C], F32R, isOutput=False)
    w3_d = nc.declare_dram_parameter("W3T", [C, C], F32R, isOutput=False)
    vp_d = nc.declare_dram_parameter("vp", [128, 24], F32, isOutput=False)
    selT_d = nc.declare_dram_parameter("selT", [GPT, 128], F32, isOutput=False)
    out_d = nc.declare_dram_parameter("out", [C, NQ], F32, isOutput=True)

    dram = dict(x=x_d, K2=k2_d, W3T=w3_d, vp=vp_d, selT=selT_d, out=out_d)
    with tile.TileContext(nc) as tc, \
         nc.allow_low_precision(reason="float32r tiles are 4-byte fp32 feeding the PE"):
        _emit(tc, {k: v.ap() for k, v in dram.items()})
    nc.compile()
    return nc


def _emit(tc, d):
    nc = tc.nc

    # ---- long-lived pools -------------------------------------------------
    xp = tc.alloc_tile_pool(name="xp", bufs=NT)
    k2p = tc.alloc_tile_pool(name="k2p", bufs=NT)
    w3p = tc.alloc_tile_pool(name="w3p", bufs=NT)
    vecs = tc.alloc_tile_pool(name="vecs", bufs=1)
    utp = tc.alloc_tile_pool(name="utp", bufs=JT)
    xqp = tc.alloc_tile_pool(name="xqp", bufs=NT)

    # ones tile via memset: warm-up lhsT + softmax-sum matmuls, no DMA.
    ones32_sb = vecs.tile([128, 128], F32, tag="ones32")
    nc.vector.memset(ones32_sb[:, :], 1.0)
    ones128_sb = vecs.tile([128, 128], F32R, tag="ones128")
    nc.vector.tensor_copy(out=ones128_sb[:, :], in_=ones32_sb[:, :])

    # ---- DMA in (sync-queue FIFO: tiny first, then x, then fused weights) -
    vp_sb = vecs.tile([128, 24], F32, tag="vp")
    nc.sync.dma_start(out=vp_sb[:, :], in_=d["vp"])
    selT_sb = vecs.tile([GPT, 128], F32, tag="selT")
    nc.sync.dma_start(out=selT_sb[:, :], in_=d["selT"])

    gnw_sb = vp_sb[:, 0:NT]
    gnb_sb = vp_sb[:, NT:2 * NT]
    wkbq_sb = vp_sb[:, 2 * NT:3 * NT]
    wobv_sb = vp_sb[:, 3 * NT:4 * NT]
    sel_sb = vp_sb[:, 4 * NT:4 * NT + GPT]

    x_sb = []
    x_t = d["x"].rearrange("(t p) n -> t p n", p=128)
    xsplits = []
    for t in range(NT):
        xt = xp.tile([128, N], F32R, tag="x", name=f"xt{t}")
        nsp = 2 if t < NT - 1 else 4   # last tile lands in quarters
        w = N // nsp
        for hh in range(nsp):
            nc.sync.dma_start(out=xt[:, hh * w:(hh + 1) * w],
                              in_=x_t[t][:, hh * w:(hh + 1) * w])
        xsplits.append([(hh * w, (hh + 1) * w) for hh in range(nsp)])
        x_sb.append(xt)

    def load_w(pool, name, tag):
        tiles = []
        r = d[name].rearrange("(t p) m -> t p m", p=128)
        for t in range(NT):
            wt = pool.tile([128, C], F32R, tag=tag)
            nc.sync.dma_start(out=wt[:, :], in_=r[t])
            tiles.append(wt)
        return tiles

    k2_sb = load_w(k2p, "K2", "K2")    # [a_part, b] raw; A-scaled in place later
    w3_sb = load_w(w3p, "W3T", "W3T")  # [b_part, co] raw; A-scaled in place later

    A_sb = vecs.tile([128, NT], F32, tag="A")
    B_sb = vecs.tile([128, NT], F32, tag="B")
    cbA_sb = vecs.tile([128, NT], F32, tag="cbA")
    wkbqA_sb = vecs.tile([128, NT], F32, tag="wkbqA")
    fb_sb = vecs.tile([128, NT], F32, tag="fb")

    # ---- GroupNorm stats -> per-channel affine A, B -----------------------
    with tc.tile_pool(name="stp", bufs=4) as stp, \
         tc.tile_pool(name="pssm", bufs=2, space="PSUM") as ps_sm:
        nwarm = [0]

        def emit_warm(n, rhs=None):
            for _ in range(n):
                wt = ps_sm.tile([128, 128], F32, tag="warm", name=f"wm{nwarm[0]}")
                nwarm[0] += 1
                nc.tensor.matmul(out=wt[:, :], lhsT=ones128_sb[:, :],
                                 rhs=ones128_sb[:, :] if rhs is None else rhs,
                                 start=True, stop=True)

        emit_warm(16)
        gps_t = []
        st2_t = []
        for t in range(NT):
            st = stp.tile([128, 8, 6], F32, tag="bnst", name=f"bnst{t}")
            xr = _f32(x_sb[t][:, :]).rearrange("p (s n) -> p s n", s=8)
            for lo, hi in xsplits[t]:
                # DVE bn_stats per 512-col chunk as each DMA piece lands;
                # a PE matmul reading the piece re-arms the HAM clock gate.
                emit_warm(1, rhs=x_sb[t][:, hi - 128:hi])
                for s in range(lo // 512, hi // 512):
                    nc.vector.bn_stats(out=st[:, s, :], in_=xr[:, s, :])
            mv = stp.tile([128, 2], F32, tag="mv", name=f"mv{t}")
            nc.vector.bn_aggr(out=mv[:, :], in_=st[:, :, :])
            st2 = stp.tile([128, 2], F32, tag="st2", name=f"st2_{t}")
            nc.vector.tensor_copy(out=st2[:, 0:1], in_=mv[:, 0:1])
            nc.vector.tensor_mul(out=st2[:, 1:2], in0=mv[:, 0:1], in1=mv[:, 0:1])
            nc.vector.tensor_add(out=st2[:, 1:2], in0=st2[:, 1:2], in1=mv[:, 1:2])
            st2_t.append(st2)
            gps = ps_sm.tile([GPT, 2], F32, tag="gps", name=f"gps{t}")
            nc.tensor.matmul(out=gps[:, :], lhsT=sel_sb, rhs=st2[:, :],
                             start=True, stop=True)
            gps_t.append(gps)

        # group mean / rstd; DVE preps first, then batched ACT sqrts
        grp_t = []
        for t in range(NT):
            grp = stp.tile([GPT, 2], F32, tag="grp", name=f"grp{t}")
            nc.vector.tensor_scalar_mul(out=grp[:, :], in0=gps_t[t][:, :], scalar1=GDIV)
            gtmp = stp.tile([GPT, 1], F32, tag="gtmp", name=f"gtmp{t}")
            nc.vector.tensor_mul(out=gtmp[:, :], in0=grp[:, 0:1], in1=grp[:, 0:1])
            nc.vector.tensor_sub(out=grp[:, 1:2], in0=grp[:, 1:2], in1=gtmp[:, :])
            nc.vector.tensor_scalar_add(out=grp[:, 1:2], in0=grp[:, 1:2], scalar1=EPS)
            grp_t.append(grp)
        for t in range(NT):
            nc.scalar.activation(out=grp_t[t][:, 1:2], in_=grp_t[t][:, 1:2],
                                 func=AF.Sqrt, bias=0.0, scale=1.0)
        for t in range(NT):
            nc.vector.reciprocal(out=grp_t[t][:, 1:2], in_=grp_t[t][:, 1:2])
            mrp = ps_sm.tile([128, 2], F32, tag="sm", name=f"mrp{t}")
            nc.tensor.matmul(out=mrp[:, :], lhsT=selT_sb[:, :], rhs=grp_t[t][:, :],
                             start=True, stop=True)
            tcol = slice(t, t + 1)
            nc.vector.tensor_mul(out=A_sb[:, tcol], in0=gnw_sb[:, tcol], in1=mrp[:, 1:2])
            nc.vector.tensor_mul(out=B_sb[:, tcol], in0=mrp[:, 0:1], in1=A_sb[:, tcol])
            nc.vector.tensor_sub(out=B_sb[:, tcol], in0=gnb_sb[:, tcol], in1=B_sb[:, tcol])

    ps_mm = tc.alloc_tile_pool(name="psmm", bufs=3, space="PSUM")

    # ---- qk bias cb = K2^T B + wk^T bq (needs raw K2, so before scaling) --
    nc.vector.tensor_mul(out=wkbqA_sb[:, :], in0=A_sb[:, :], in1=wkbq_sb)
    for bb in range(NT):
        bps = ps_mm.tile([128, 1], F32, tag="mm", name=f"cb{bb}")
        for a in range(NT):
            nc.tensor.matmul(out=bps[:, :],
                             lhsT=_f32(k2_sb[a][:, bb * 128:(bb + 1) * 128]),
                             rhs=B_sb[:, a:a + 1],
                             start=(a == 0), stop=(a == NT - 1))
        # cbA = A*(cb_psum) + A*wkbq
        nc.vector.tensor_scalar(out=cbA_sb[:, bb:bb + 1], in0=bps[:, :],
                                scalar1=A_sb[:, bb:bb + 1],
                                scalar2=wkbqA_sb[:, bb:bb + 1],
                                op0=ALU.mult, op1=ALU.add)

    # ---- K2A = A (.) K2 in place, then qk2 chunk 0 ------------------------
    for a in range(NT):
        nc.vector.tensor_scalar_mul(out=k2_sb[a][:, :], in0=_f32(k2_sb[a][:, :]),
                                    scalar1=A_sb[:, a:a + 1])

    qkp = tc.alloc_tile_pool(name="qkp", bufs=NT)

    def emit_qk(ch):
        csl = slice(ch * CW, (ch + 1) * CW)
        qk2 = []
        for bb in range(NT):
            qps = ps_mm.tile([128, CW], F32, tag="mm")
            for a in range(NT):
                nc.tensor.matmul(out=qps[:, :],
                                 lhsT=k2_sb[a][:, bb * 128:(bb + 1) * 128],
                                 rhs=x_sb[a][:, csl],
                                 start=(a == 0), stop=(a == NT - 1))
            qk = qkp.tile([128, CW], F32R, tag="qk")
            nc.vector.tensor_scalar(out=qk[:, :], in0=qps[:, :],
                                    scalar1=A_sb[:, bb:bb + 1],
                                    scalar2=cbA_sb[:, bb:bb + 1],
                                    op0=ALU.mult, op1=ALU.add)
            qk2.append(qk)
        return qk2

    qk2_ch = emit_qk(0)

    # ---- out bias fb = W3T^T B + (wo@bv + bo) (raw W3T, before scaling) ---
    for cob in range(NT):
        fps = ps_mm.tile([128, 1], F32, tag="mm", name=f"fb{cob}")
        for b in range(NT):
            nc.tensor.matmul(out=fps[:, :],
                             lhsT=_f32(w3_sb[b][:, cob * 128:(cob + 1) * 128]),
                             rhs=B_sb[:, b:b + 1],
                             start=(b == 0), stop=(b == NT - 1))
        nc.vector.tensor_add(out=fb_sb[:, cob:cob + 1], in0=fps[:, :],
                             in1=wobv_sb[:, cob:cob + 1])

    # ---- W3AT = A (.) W3T in place ----------------------------------------
    for b in range(NT):
        nc.vector.tensor_scalar_mul(out=w3_sb[b][:, :], in0=_f32(w3_sb[b][:, :]),
                                    scalar1=A_sb[:, b:b + 1])

    # xq = x[:, 0:NQ] + fb on GpSimd (off the DVE critical path)
    xq_sb = []
    for co in range(NT):
        xq = xqp.tile([128, NQ], F32, tag="xq", name=f"xq{co}")
        for h in range(NCH):
            sl = slice(h * CW, (h + 1) * CW)
            nc.gpsimd.tensor_scalar_add(out=xq[:, sl], in0=_f32(x_sb[co][:, sl]),
                                        scalar1=fb_sb[:, co:co + 1])
        xq_sb.append(xq)

    # ---- attention chunks -------------------------------------------------
    ps_o = tc.alloc_tile_pool(name="pso", bufs=NT, space="PSUM")
    pp = tc.alloc_tile_pool(name="pp", bufs=5)
    outp = tc.alloc_tile_pool(name="outp", bufs=4)
    smsb = tc.alloc_tile_pool(name="smsb", bufs=2)

    ut_sb = []

    for ch in range(NCH):
        csl = slice(ch * CW, (ch + 1) * CW)
        if ch > 0:
            qk2_ch = emit_qk(ch)

        o_ps = [ps_o.tile([128, CW], F32, tag="o", name=f"o{ch}_{i}") for i in range(4)]
        sacc = smsb.tile([128, CW], F32R, tag="sacc", name=f"sacc{ch}")
        P_t = [None] * JT
        for jt in range(JT):
            jsl = slice(jt * 128, (jt + 1) * 128)
            lps = ps_mm.tile([128, CW], F32, tag="mm")
            for b in range(NT):
                nc.tensor.matmul(out=lps[:, :], lhsT=x_sb[b][:, jsl],
                                 rhs=qk2_ch[b][:, :],
                                 start=(b == 0), stop=(b == NT - 1))
            P = pp.tile([128, CW], F32R, tag="P")
            nc.scalar.activation(out=P[:, :], in_=lps[:, :], func=AF.Exp,
                                 bias=0.0, scale=SCALE)
            P_t[jt] = P
            if ch == 0:
                # ut[jt] = (A.W3T)^T x — between lps and the lagged o mms,
                # also hides the exp latency
                ups = ps_mm.tile([128, C], F32, tag="mm")
                for b in range(NT):
                    nc.tensor.matmul(out=ups[:, :], lhsT=x_sb[b][:, jsl],
                                     rhs=w3_sb[b][:, :],
                                     start=(b == 0), stop=(b == NT - 1))
                ut = utp.tile([128, C], F32R, tag="ut")
                nc.scalar.activation(out=ut[:, :], in_=ups[:, :], func=AF.Copy,
                                     bias=0.0, scale=1.0)
                ut_sb.append(ut)
            # o mms lag one iteration: P[jt-1] is ready, no ACT stall
            if jt > 0:
                for co in range(4):
                    nc.tensor.matmul(out=o_ps[co][:, :],
                                     lhsT=ut_sb[jt - 1][:, co * 128:(co + 1) * 128],
                                     rhs=P_t[jt - 1][:, :],
                                     start=(jt == 1), stop=False,
                                     skip_group_check=True)
            # running softmax denominator on DVE (jt<=30; P31 via matmul)
            if jt == 0:
                nc.vector.tensor_copy(out=sacc[:, :], in_=_f32(P[:, :]))
            elif jt < JT - 1:
                nc.vector.tensor_add(out=sacc[:, :], in0=_f32(sacc[:, :]),
                                     in1=_f32(P[:, :]))

        # 1/s: s = ones@sacc + ones@P31, ready before the last o mms finish
        rbp = ps_mm.tile([128, CW], F32, tag="mm")
        nc.tensor.matmul(out=rbp[:, :], lhsT=ones128_sb[:, :], rhs=sacc[:, :],
                         start=True, stop=False)
        nc.tensor.matmul(out=rbp[:, :], lhsT=ones128_sb[:, :], rhs=P_t[JT - 1][:, :],
                         start=False, stop=True)
        rsb = smsb.tile([128, CW], F32, tag="rsb")
        nc.vector.reciprocal_approx_fast(out=rsb[:, :], in_=rbp[:, :])
        for co in range(4):
            nc.tensor.matmul(out=o_ps[co][:, :],
                             lhsT=ut_sb[JT - 1][:, co * 128:(co + 1) * 128],
                             rhs=P_t[JT - 1][:, :],
                             start=False, stop=True, skip_group_check=True)

        # epilogue: DVE normalizes (frees the PSUM bank), GpSimd adds x+fb
        for co in range(4):
            ot_ = outp.tile([128, CW], F32, tag="osb")
            nc.vector.tensor_mul(out=ot_[:, :], in0=o_ps[co][:, :], in1=rsb[:, :])
            ou = outp.tile([128, CW], F32, tag="osb")
            nc.gpsimd.tensor_add(out=ou[:, :], in0=ot_[:, :],
                                 in1=xq_sb[co][:, csl])
            nc.sync.dma_start(out=d["out"][co * 128:(co + 1) * 128, csl], in_=ou[:, :])

    for p in (smsb, outp, pp, ps_o, qkp, ps_mm, xqp, utp, vecs, w3p, k2p, xp):
        p.release()


def _sel_consts():
    sel = np.zeros((128, GPT), np.float32)
    for p in range(128):
        sel[p, p // 16] = 1.0
    return sel, np.ascontiguousarray(sel.T)


def kernel(x, gn_w, gn_b, wq, bq, wk, bk, wv, bv, wo, bo):
    del bk  # exactly cancelled by softmax shift invariance
    if "nc" not in _CACHE:
        _CACHE["nc"] = _build_bass()
    nc = _CACHE["nc"]

    x = np.ascontiguousarray(np.asarray(x, np.float32)).reshape(B, C, N)
    wq64 = np.asarray(wq, np.float64)
    wk64 = np.asarray(wk, np.float64)
    wv64 = np.asarray(wv, np.float64)
    wo64 = np.asarray(wo, np.float64)
    K2 = np.ascontiguousarray((wq64.T @ wk64).astype(np.float32))
    W3T = np.ascontiguousarray((wo64 @ wv64).T.astype(np.float32))
    wkbq = (wk64.T @ np.asarray(bq, np.float64)).astype(np.float32)
    wobvbo = (wo64 @ np.asarray(bv, np.float64)
              + np.asarray(bo, np.float64)).astype(np.float32)
    sel, selT = _sel_consts()
    vp = np.concatenate([
        np.asarray(gn_w, np.float32).reshape(NT, 128).T,
        np.asarray(gn_b, np.float32).reshape(NT, 128).T,
        wkbq.reshape(NT, 128).T,
        wobvbo.reshape(NT, 128).T,
        sel,
    ], axis=1)
    vp = np.ascontiguousarray(vp)

    in_maps = []
    for core in range(8):
        b, qb = core // 4, core % 4
        xb = np.ascontiguousarray(np.roll(x[b], -qb * NQ, axis=1))
        in_maps.append({"x": xb, "K2": K2, "W3T": W3T, "vp": vp, "selT": selT})

    _CACHE["last_in_maps"] = in_maps
    res = run_bass_kernel_spmd(nc, in_maps, list(range(8))).results
    out = np.empty((B, C, N), np.float32)
    for core in range(8):
        b, qb = core // 4, core % 4
        out[b][:, qb * NQ:(qb + 1) * NQ] = res[core]["out"]
    return out.reshape(B, C, HH, WW)


# revision 8
# speedup vs baseline: 1.1735x; 1.1397x over previous
"""AttnBlock (GroupNorm + single-head self-attention + proj + residual) on 8 trn2 cores.

Sharding: core = (batch b = core//4, query-block qb = core%4). Each core gets its
batch's x rolled so its 1024 queries are columns 0:1024; attention key/value
order is permutation-invariant so the roll is free. No cross-core communication.

Math (validated in numpy to 7e-8 rel err): the four 1x1 convs are fused on the
HOST into two C*C matrices (pure weight preprocessing, no data dependence):
    K2  = wq^T @ wk          (logits bilinear form:  l[i,j] = hn_i^T K2 hn_j)
    W3  = wo @ wv            (value+proj fused:      out_pre = W3 @ hn @ P)
GroupNorm folds into a per-channel affine hn = A*x + B on device:
    qk2[b,i] = A[b] * (sum_a A[a] K2[a,b] x[a,i]) + A[b]*cb[b],
      cb = K2^T B + wk^T bq;  the pure-B logits terms are constant per query
      and cancel in softmax, exactly like the k-bias.
    logitsT[j,i] = sum_b x[b,j] qk2[b,i]   (keys-major, no transposes)
    P = exp(logitsT/sqrt(C)) unnormalized; o_pre = ut^T @ P with
    ut[j,co] = sum_b A[b] W3T[b,co] x[b,j]; out = o_pre/s + fb + x where
    fb = W3^T... (W3T^T B + wo@bv + bo) and s = column sums of P.

Schedule (from perfetto analysis of the 262us baseline):
  - DMA order: vp/selT (tiny), x (10 pieces), K2, W3T. All on the sync queue
    FIFO; x lands ~21us, stats chain done ~25us, main loop starts ~28us.
  - PE warm-up matmuls on a memset ones tile start at ~0.8us (no DMA!) and are
    re-armed by matmuls reading each landed x piece, so the HAM clock gate
    never sees a 3.4us idle window (which would halve the PE clock).
  - Main loop per key tile: 4 logits mm, exp on ACT, 4 ut mm (chunk 0 only),
    4 o mms lagged one iteration behind (hides exp latency), running-sum of P
    on DVE, ut PSUM->SBUF drain on ACT.
  - Softmax denominator: sum P(jt<=30) on DVE + final P31 via a second
    accumulating ones-matmul, so 1/s is ready before the last o mm finishes.
  - Epilogue per chunk: DVE mul by 1/s (frees PSUM banks asap), residual+bias
    add on GpSimd (xq = x + fb precomputed there too), DMA out per 128-row
    block as it completes.
"""

import numpy as np

import concourse.bass as bass
import concourse.bacc as bacc
import concourse.tile as tile
from concourse import mybir
from concourse.bass_utils import run_bass_kernel_spmd

F32 = mybir.dt.float32
F32R = mybir.dt.float32r
AF = mybir.ActivationFunctionType
ALU = mybir.AluOpType
AX = mybir.AxisListType

B, C, HH, WW = 2, 512, 64, 64
N = HH * WW          # 4096 pixels
NQ = N // 4          # queries per core
G = 32               # groups
GPT = 8              # groups per 128-channel tile
NT = C // 128        # 4 channel tiles
JT = N // 128        # 32 key tiles
CW = 512             # query chunk width
NCH = NQ // CW       # 2 chunks per core
EPS = 1e-6
SCALE = float(C) ** -0.5
GDIV = 1.0 / 16.0    # 16 channels per group

_CACHE: dict = {}


def _f32(ap):
    return ap.bitcast(F32)


def _build_bass():
    nc = bacc.Bacc("TRN2")

    x_d = nc.declare_dram_parameter("x", [C, N], F32R, isOutput=False)
    k2_d = nc.declare_dram_parameter("K2", [C, C], F32R, isOutput=False)
    w3_d = nc.declare_dram_parameter("W3T", [C, C], F32R, isOutput=False)
    vp_d = nc.declare_dram_parameter("vp", [128, 24], F32, isOutput=False)
    selT_d = nc.declare_dram_parameter("selT", [GPT, 128], F32, isOutput=False)
    out_d = nc.declare_dram_parameter("out", [C, NQ], F32, isOutput=True)

    dram = dict(x=x_d, K2=k2_d, W3T=w3_d, vp=vp_d, selT=selT_d, out=out_d)
    with tile.TileContext(nc) as tc, \
         nc.allow_low_precision(reason="float32r tiles are 4-byte fp32 feeding the PE"):
        _emit(tc, {k: v.ap() for k, v in dram.items()})
    nc.compile()
    return nc


def _emit(tc, d):
    nc = tc.nc

    # ---- long-lived pools -------------------------------------------------
    xp = tc.alloc_tile_pool(name="xp", bufs=NT)
    k2p = tc.alloc_tile_pool(name="k2p", bufs=NT)
    w3p = tc.alloc_tile_pool(name="w3p", bufs=NT)
    vecs = tc.alloc_tile_pool(name="vecs", bufs=1)
    utp = tc.alloc_tile_pool(name="utp", bufs=JT)
    xqp = tc.alloc_tile_pool(name="xqp", bufs=NT)

    # ones tile via memset: warm-up lhsT + softmax-sum matmuls, no DMA.
    ones32_sb = vecs.tile([128, 128], F32, tag="ones32")
    nc.vector.memset(ones32_sb[:, :], 1.0)
    ones128_sb = vecs.tile([128, 128], F32R, tag="ones128")
    nc.vector.tensor_copy(out=ones128_sb[:, :], in_=ones32_sb[:, :])

    # ---- DMA in (sync-queue FIFO: tiny first, then x, then fused weights) -
    vp_sb = vecs.tile([128, 24], F32, tag="vp")
    nc.sync.dma_start(out=vp_sb[:, :], in_=d["vp"])
    selT_sb = vecs.tile([GPT, 128], F32, tag="selT")
    nc.sync.dma_start(out=selT_sb[:, :], in_=d["selT"])

    gnw_sb = vp_sb[:, 0:NT]
    gnb_sb = vp_sb[:, NT:2 * NT]
    wkbq_sb = vp_sb[:, 2 * NT:3 * NT]
    wobv_sb = vp_sb[:, 3 * NT:4 * NT]
    sel_sb = vp_sb[:, 4 * NT:4 * NT + GPT]

    x_sb = []
    x_t = d["x"].rearrange("(t p) n -> t p n", p=128)
    xsplits = []
    for t in range(NT):
        xt = xp.tile([128, N], F32R, tag="x", name=f"xt{t}")
        nsp = 2 if t < NT - 1 else 4   # last tile lands in quarters
        w = N // nsp
        for hh in range(nsp):
            nc.sync.dma_start(out=xt[:, hh * w:(hh + 1) * w],
                              in_=x_t[t][:, hh * w:(hh + 1) * w])
        xsplits.append([(hh * w, (hh + 1) * w) for hh in range(nsp)])
        x_sb.append(xt)

    def load_w(pool, name, tag):
        tiles = []
        r = d[name].rearrange("(t p) m -> t p m", p=128)
        for t in range(NT):
            wt = pool.tile([128, C], F32R, tag=tag)
            nc.sync.dma_start(out=wt[:, :], in_=r[t])
            tiles.append(wt)
        return tiles

    k2_sb = load_w(k2p, "K2", "K2")    # [a_part, b] raw; A-scaled in place later
    w3_sb = load_w(w3p, "W3T", "W3T")  # [b_part, co] raw; A-scaled in place later

    A_sb = vecs.tile([128, NT], F32, tag="A")
    B_sb = vecs.tile([128, NT], F32, tag="B")
    cbA_sb = vecs.tile([128, NT], F32, tag="cbA")
    wkbqA_sb = vecs.tile([128, NT], F32, tag="wkbqA")
    fb_sb = vecs.tile([128, NT], F32, tag="fb")

    # ---- GroupNorm stats -> per-channel affine A, B -----------------------
    with tc.tile_pool(name="stp", bufs=4) as stp, \
         tc.tile_pool(name="pssm", bufs=2, space="PSUM") as ps_sm:
        nwarm = [0]

        def emit_warm(n, rhs=None):
            for _ in range(n):
                wt = ps_sm.tile([128, 128], F32, tag="warm", name=f"wm{nwarm[0]}")
                nwarm[0] += 1
                nc.tensor.matmul(out=wt[:, :], lhsT=ones128_sb[:, :],
                                 rhs=ones128_sb[:, :] if rhs is None else rhs,
                                 start=True, stop=True)

        emit_warm(16)
        gps_t = []
        st2_t = []
        for t in range(NT):
            st = stp.tile([128, 8, 6], F32, tag="bnst", name=f"bnst{t}")
            xr = _f32(x_sb[t][:, :]).rearrange("p (s n) -> p s n", s=8)
            for lo, hi in xsplits[t]:
                # DVE bn_stats per 512-col chunk as each DMA piece lands;
                # a PE matmul reading the piece re-arms the HAM clock gate.
                emit_warm(1, rhs=x_sb[t][:, hi - 128:hi])
                for s in range(lo // 512, hi // 512):
                    nc.vector.bn_stats(out=st[:, s, :], in_=xr[:, s, :])
            mv = stp.tile([128, 2], F32, tag="mv", name=f"mv{t}")
            nc.vector.bn_aggr(out=mv[:, :], in_=st[:, :, :])
            st2 = stp.tile([128, 2], F32, tag="st2", name=f"st2_{t}")
            nc.vector.tensor_copy(out=st2[:, 0:1], in_=mv[:, 0:1])
            nc.vector.tensor_mul(out=st2[:, 1:2], in0=mv[:, 0:1], in1=mv[:, 0:1])
            nc.vector.tensor_add(out=st2[:, 1:2], in0=st2[:, 1:2], in1=mv[:, 1:2])
            st2_t.append(st2)
            gps = ps_sm.tile([GPT, 2], F32, tag="gps", name=f"gps{t}")
            nc.tensor.matmul(out=gps[:, :], lhsT=sel_sb, rhs=st2[:, :],
                             start=True, stop=True)
            gps_t.append(gps)

        # group mean / rstd; DVE preps first, then batched ACT sqrts
        grp_t = []
        for t in range(NT):
            grp = stp.tile([GPT, 2], F32, tag="grp", name=f"grp{t}")
            nc.vector.tensor_scalar_mul(out=grp[:, :], in0=gps_t[t][:, :], scalar1=GDIV)
            gtmp = stp.tile([GPT, 1], F32, tag="gtmp", name=f"gtmp{t}")
            nc.vector.tensor_mul(out=gtmp[:, :], in0=grp[:, 0:1], in1=grp[:, 0:1])
            nc.vector.tensor_sub(out=grp[:, 1:2], in0=grp[:, 1:2], in1=gtmp[:, :])
            nc.vector.tensor_scalar_add(out=grp[:, 1:2], in0=grp[:, 1:2], scalar1=EPS)
            grp_t.append(grp)
        for t in range(NT):
            nc.scalar.activation(out=grp_t[t][:, 1:2], in_=grp_t[t][:, 1:2],
                                 func=AF.Sqrt, bias=0.0, scale=1.0)
        for t in range(NT):
            nc.vector.reciprocal(out=grp_t[t][:, 1:2], in_=grp_t[t][:, 1:2])
            mrp = ps_sm.tile([128, 2], F32, tag="sm", name=f"mrp{t}")
            nc.tensor.matmul(out=mrp[:, :], lhsT=selT_sb[:, :], rhs=grp_t[t][:, :],
                             start=True, stop=True)
            tcol = slice(t, t + 1)
            nc.vector.tensor_mul(out=A_sb[:, tcol], in0=gnw_sb[:, tcol], in1=mrp[:, 1:2])
            nc.vector.tensor_mul(out=B_sb[:, tcol], in0=mrp[:, 0:1], in1=A_sb[:, tcol])
            nc.vector.tensor_sub(out=B_sb[:, tcol], in0=gnb_sb[:, tcol], in1=B_sb[:, tcol])

    ps_mm = tc.alloc_tile_pool(name="psmm", bufs=3, space="PSUM")

    # ---- qk bias cb = K2^T B + wk^T bq (needs raw K2, so before scaling) --
    nc.vector.tensor_mul(out=wkbqA_sb[:, :], in0=A_sb[:, :], in1=wkbq_sb)
    for bb in range(NT):
        bps = ps_mm.tile([128, 1], F32, tag="mm", name=f"cb{bb}")
        for a in range(NT):
            nc.tensor.matmul(out=bps[:, :],
                             lhsT=_f32(k2_sb[a][:, bb * 128:(bb + 1) * 128]),
                             rhs=B_sb[:, a:a + 1],
                             start=(a == 0), stop=(a == NT - 1))
        # cbA = A*(cb_psum) + A*wkbq
        nc.vector.tensor_scalar(out=cbA_sb[:, bb:bb + 1], in0=bps[:, :],
                                scalar1=A_sb[:, bb:bb + 1],
                                scalar2=wkbqA_sb[:, bb:bb + 1],
                                op0=ALU.mult, op1=ALU.add)

    # ---- K2A = A (.) K2 in place, then qk2 chunk 0 ------------------------
    for a in range(NT):
        nc.vector.tensor_scalar_mul(out=k2_sb[a][:, :], in0=_f32(k2_sb[a][:, :]),
                                    scalar1=A_sb[:, a:a + 1])

    qkp = tc.alloc_tile_pool(name="qkp", bufs=NT)

    def emit_qk(ch):
        csl = slice(ch * CW, (ch + 1) * CW)
        qk2 = []
        for bb in range(NT):
            qps = ps_mm.tile([128, CW], F32, tag="mm")
            for a in range(NT):
                nc.tensor.matmul(out=qps[:, :],
                                 lhsT=k2_sb[a][:, bb * 128:(bb + 1) * 128],
                                 rhs=x_sb[a][:, csl],
                                 start=(a == 0), stop=(a == NT - 1))
            qk = qkp.tile([128, CW], F32R, tag="qk")
            nc.vector.tensor_scalar(out=qk[:, :], in0=qps[:, :],
                                    scalar1=A_sb[:, bb:bb + 1],
                                    scalar2=cbA_sb[:, bb:bb + 1],
                                    op0=ALU.mult, op1=ALU.add)
            qk2.append(qk)
        return qk2

    qk2_ch = emit_qk(0)

    # ---- out bias fb = W3T^T B + (wo@bv + bo) (raw W3T, before scaling) ---
    for cob in range(NT):
        fps = ps_mm.tile([128, 1], F32, tag="mm", name=f"fb{cob}")
        for b in range(NT):
            nc.tensor.matmul(out=fps[:, :],
                             lhsT=_f32(w3_sb[b][:, cob * 128:(cob + 1) * 128]),
                             rhs=B_sb[:, b:b + 1],
                             start=(b == 0), stop=(b == NT - 1))
        nc.vector.tensor_add(out=fb_sb[:, cob:cob + 1], in0=fps[:, :],
                             in1=wobv_sb[:, cob:cob + 1])

    # ---- W3AT = A (.) W3T in place ----------------------------------------
    for b in range(NT):
        nc.vector.tensor_scalar_mul(out=w3_sb[b][:, :], in0=_f32(w3_sb[b][:, :]),
                                    scalar1=A_sb[:, b:b + 1])

    # xq = x[:, 0:NQ] + fb (DVE; GpSimd is ~10x slower and steals SBUF ports)
    xq_sb = []
    for co in range(NT):
        xq = xqp.tile([128, NQ], F32, tag="xq", name=f"xq{co}")
        for h in range(NCH):
            sl = slice(h * CW, (h + 1) * CW)
            nc.vector.tensor_scalar_add(out=xq[:, sl], in0=_f32(x_sb[co][:, sl]),
                                        scalar1=fb_sb[:, co:co + 1])
        xq_sb.append(xq)

    # ---- attention chunks -------------------------------------------------
    ps_o = tc.alloc_tile_pool(name="pso", bufs=NT, space="PSUM")
    pp = tc.alloc_tile_pool(name="pp", bufs=5)
    outp = tc.alloc_tile_pool(name="outp", bufs=4)
    smsb = tc.alloc_tile_pool(name="smsb", bufs=2)

    ut_sb = []

    for ch in range(NCH):
        csl = slice(ch * CW, (ch + 1) * CW)
        if ch > 0:
            qk2_ch = emit_qk(ch)

        o_ps = [ps_o.tile([128, CW], F32, tag="o", name=f"o{ch}_{i}") for i in range(4)]
        sacc = smsb.tile([128, CW], F32R, tag="sacc", name=f"sacc{ch}")
        P_t = [None] * JT
        for jt in range(JT):
            jsl = slice(jt * 128, (jt + 1) * 128)
            lps = ps_mm.tile([128, CW], F32, tag="mm")
            for b in range(NT):
                nc.tensor.matmul(out=lps[:, :], lhsT=x_sb[b][:, jsl],
                                 rhs=qk2_ch[b][:, :],
                                 start=(b == 0), stop=(b == NT - 1))
            P = pp.tile([128, CW], F32R, tag="P")
            nc.scalar.activation(out=P[:, :], in_=lps[:, :], func=AF.Exp,
                                 bias=0.0, scale=SCALE)
            P_t[jt] = P
            if ch == 0:
                # ut[jt] = (A.W3T)^T x — between lps and the lagged o mms,
                # also hides the exp latency
                ups = ps_mm.tile([128, C], F32, tag="mm")
                for b in range(NT):
                    nc.tensor.matmul(out=ups[:, :], lhsT=x_sb[b][:, jsl],
                                     rhs=w3_sb[b][:, :],
                                     start=(b == 0), stop=(b == NT - 1))
                ut = utp.tile([128, C], F32R, tag="ut")
                nc.scalar.activation(out=ut[:, :], in_=ups[:, :], func=AF.Copy,
                                     bias=0.0, scale=1.0)
                ut_sb.append(ut)
            # o mms lag one iteration: P[jt-1] is ready, no ACT stall
            if jt > 0:
                for co in range(4):
                    nc.tensor.matmul(out=o_ps[co][:, :],
                                     lhsT=ut_sb[jt - 1][:, co * 128:(co + 1) * 128],
                                     rhs=P_t[jt - 1][:, :],
                                     start=(jt == 1), stop=False,
                                     skip_group_check=True)
            # running softmax denominator on DVE (jt<=30; P31 via matmul)
            if jt == 0:
                nc.vector.tensor_copy(out=sacc[:, :], in_=_f32(P[:, :]))
            elif jt < JT - 1:
                nc.vector.tensor_add(out=sacc[:, :], in0=_f32(sacc[:, :]),
                                     in1=_f32(P[:, :]))

        # 1/s: s = ones@sacc + ones@P31, ready before the last o mms finish
        rbp = ps_mm.tile([128, CW], F32, tag="mm")
        nc.tensor.matmul(out=rbp[:, :], lhsT=ones128_sb[:, :], rhs=sacc[:, :],
                         start=True, stop=False)
        nc.tensor.matmul(out=rbp[:, :], lhsT=ones128_sb[:, :], rhs=P_t[JT - 1][:, :],
                         start=False, stop=True)
        rsb = smsb.tile([128, CW], F32, tag="rsb")
        nc.vector.reciprocal_approx_fast(out=rsb[:, :], in_=rbp[:, :])
        for co in range(4):
            nc.tensor.matmul(out=o_ps[co][:, :],
                             lhsT=ut_sb[JT - 1][:, co * 128:(co + 1) * 128],
                             rhs=P_t[JT - 1][:, :],
                             start=False, stop=True, skip_group_check=True)

        # epilogue: DVE muls first (frees all PSUM banks asap), then adds
        ot_t = []
        for co in range(4):
            ot_ = outp.tile([128, CW], F32, tag="osb", name=f"n{ch}_{co}")
            nc.vector.tensor_mul(out=ot_[:, :], in0=o_ps[co][:, :], in1=rsb[:, :])
            ot_t.append(ot_)
        for co in range(4):
            ou = outp.tile([128, CW], F32, tag="oadd", name=f"r{ch}_{co}")
            nc.vector.tensor_add(out=ou[:, :], in0=ot_t[co][:, :],
                                 in1=xq_sb[co][:, csl])
            nc.sync.dma_start(out=d["out"][co * 128:(co + 1) * 128, csl], in_=ou[:, :])

    for p in (smsb, outp, pp, ps_o, qkp, ps_mm, xqp, utp, vecs, w3p, k2p, xp):
        p.release()


def _sel_consts():
    sel = np.zeros((128, GPT), np.float32)
    for p in range(128):
        sel[p, p // 16] = 1.0
    return sel, np.ascontiguousarray(sel.T)


def kernel(x, gn_w, gn_b, wq, bq, wk, bk, wv, bv, wo, bo):
    del bk  # exactly cancelled by softmax shift invariance
    if "nc" not in _CACHE:
        _CACHE["nc"] = _build_bass()
    nc = _CACHE["nc"]

    x = np.ascontiguousarray(np.asarray(x, np.float32)).reshape(B, C, N)
    wq64 = np.asarray(wq, np.float64)
    wk64 = np.asarray(wk, np.float64)
    wv64 = np.asarray(wv, np.float64)
    wo64 = np.asarray(wo, np.float64)
    K2 = np.ascontiguousarray((wq64.T @ wk64).astype(np.float32))
    W3T = np.ascontiguousarray((wo64 @ wv64).T.astype(np.float32))
    wkbq = (wk64.T @ np.asarray(bq, np.float64)).astype(np.float32)
    wobvbo = (wo64 @ np.asarray(bv, np.float64)
              + np.asarray(bo, np.float64)).astype(np.float32)
    sel, selT = _sel_consts()
    vp = np.concatenate([
        np.asarray(gn_w, np.float32).reshape(NT, 128).T,
        np.asarray(gn_b, np.float32).reshape(NT, 128).T,
        wkbq.reshape(NT, 128).T,
        wobvbo.reshape(NT, 128).T,
        sel,
    ], axis=1)
    vp = np.ascontiguousarray(vp)

    in_maps = []
    for core in range(8):
        b, qb = core // 4, core % 4
        xb = np.ascontiguousarray(np.roll(x[b], -qb * NQ, axis=1))
        in_maps.append({"x": xb, "K2": K2, "W3T": W3T, "vp": vp, "selT": selT})

    _CACHE["last_in_maps"] = in_maps
    res = run_bass_kernel_spmd(nc, in_maps, list(range(8))).results
    out = np.empty((B, C, N), np.float32)
    for core in range(8):
        b, qb = core // 4, core % 4
        out[b][:, qb * NQ:(qb + 1) * NQ] = res[core]["out"]
    return out.reshape(B, C, HH, WW)
